# revision 1
# baseline (speedup 1.0000x reference)
"""GeneSAGE (2-layer GraphSAGE + skip + LayerNorm + ELU) on 8 Trainium2 cores.

Strategy: edge-parallel by *destination range*. Core c owns nodes
[CP*c, CP*(c+1)) with CP=6272 (=49*128). Edges are bucketed host-side by
(dst-core, src-half, dst-window) for conv1 and (dst-core, dst-window) for
conv2, padded to 128-edge chunks with a chunk structure common to all 8
cores (SPMD: one program). Per chunk: one-hot(dst) built on DVE, segment
sum done as fp32 one-hot matmuls accumulating in PSUM per 128-node window
(conv1 feature-major: out = gathered^T @ onehot, so the dense phase needs
no mean transpose). Conv1 gathers x rows (256B) from HBM via dma_gather.
Node degrees (reciprocals) are precomputed on the host. The dense phase
(mean/linear/LN/ELU/p,r) is interleaved per-window under conv1's second
edge stream, with all PSUM epilogues deferred by one window to keep the
DVE/PE pipelines from lockstepping. Conv2 gathers from a 32-node-packed
p table ([npad, 2] f32 viewed as [npad/32, 64], distributed by a 50KB
per-core AllGather of the locally computed p) and extracts the 8B pair
per edge with one fused scalar_tensor_tensor; each window accumulates a
[128, 64] PSUM that is tree-reduced to [128, 2] at the end.
"""

import numpy as np

import concourse.mybir as mybir
from concourse import bacc, bass, tile
from concourse.bass_utils import run_bass_kernel_spmd

F32 = mybir.dt.float32
F32R = mybir.dt.float32r
I16 = mybir.dt.int16
I32 = mybir.dt.int32

N_CORES = 8
D = 64          # input feature dim
HID = 256
OUT = 2
LN_EPS = 1e-5
BATCH_CHUNKS = 32   # chunks per dma_gather call
USE_F32R = False
N_SWDGE_Q = 1


def _mm_cast(ap):
    return ap.bitcast(F32R) if USE_F32R else ap


def make_plan(edge_index: np.ndarray, n_nodes: int):
    """Host-side edge bucketing + degree precompute."""
    cp = int(np.ceil(n_nodes / (N_CORES * 128))) * 128
    nw = cp // 128
    npad = N_CORES * cp
    half = npad // 2
    assert half <= 32768, "int16 gather index limit"

    src = edge_index[0].astype(np.int64)
    dst = edge_index[1].astype(np.int64)
    E = src.shape[0]

    # degrees -> reciprocal of count per node, [cores, 128, nw]
    deg = np.bincount(dst, minlength=npad).astype(np.float64)
    rc = (1.0 / np.maximum(deg, 1.0)).astype(np.float32)
    rc_tile = rc.reshape(N_CORES, nw, 128).transpose(0, 2, 1).copy()

    core = dst // cp
    win = (dst % cp) // 128

    def bucket(streams, gidx_vals, extra_vals=None):
        """streams: per-edge stream id (0..S-1); returns plan piece."""
        S = int(streams.max()) + 1 if E else 1
        ngrp = S * nw
        key = (core * S + streams) * nw + win
        order = np.argsort(key, kind="stable")
        counts = np.bincount(key, minlength=N_CORES * ngrp).reshape(
            N_CORES, S, nw)
        nchunks = -(-counts.max(axis=0) // 128)  # [S, nw]
        off = np.zeros((S, nw), np.int64)
        running = 0
        for s in range(S):
            for w in range(nw):
                off[s, w] = running
                running += nchunks[s, w]
        c_total = int(running)
        e_slots = c_total * 128

        sk = key[order]
        grp_start = np.searchsorted(sk, np.arange(N_CORES * ngrp))
        rank = np.arange(E) - grp_start[sk]
        s_of = (sk // nw) % S
        w_of = sk % nw
        c_of = sk // ngrp
        slot = off[s_of, w_of] * 128 + rank

        gidx = np.zeros((N_CORES, e_slots), np.int16)
        dstf = np.full((N_CORES, e_slots), -1.0, np.float32)
        gidx[c_of, slot] = gidx_vals[order].astype(np.int16)
        dstf[c_of, slot] = (dst[order] % cp - w_of * 128).astype(np.float32)
        extra = None
        if extra_vals is not None:
            extra = np.full((N_CORES, e_slots), -1.0, np.float32)
            extra[c_of, slot] = extra_vals[order].astype(np.float32)

        a = gidx.reshape(N_CORES, e_slots // 16, 16).transpose(0, 2, 1)
        gidx_tile = np.tile(a, (1, 8, 1)).copy()  # [c, 128, J]
        dstf_tile = dstf.reshape(N_CORES, c_total, 128).transpose(0, 2, 1).copy()
        extra_tile = None
        if extra is not None:
            extra_tile = extra.reshape(N_CORES, c_total, 128).transpose(
                0, 2, 1).copy()

        sched = []
        for s in range(S):
            rows = []
            for w in range(nw):
                n = int(nchunks[s, w])
                first = int(off[s, w])
                rows.append((w, first, first + n - 1) if n else (w, -1, -2))
            sched.append(rows)
        return dict(c_total=c_total, sched=sched, gidx_tile=gidx_tile,
                    dstf_tile=dstf_tile, extra_tile=extra_tile)

    # conv1: 2 streams by src half, gather idx = src - s*half
    stream1 = (src >= half).astype(np.int64)
    p1 = bucket(stream1, src - stream1 * half)
    # conv2: single stream, gather idx = src >> 5 (32-packed p table),
    # extra = src & 31 (pair slot within the 256B row)
    p2 = bucket(np.zeros(E, np.int64), src >> 5, src & 31)

    return dict(cp=cp, nw=nw, npad=npad, half=half, p1=p1, p2=p2,
                rc_tile=rc_tile)


def build_program(plan):
    cp, nw, half, npad = plan["cp"], plan["nw"], plan["half"], plan["npad"]
    p1, p2 = plan["p1"], plan["p2"]
    c1, c2 = p1["c_total"], p2["c_total"]
    J1, J2 = c1 * 8, c2 * 8

    nc = bacc.Bacc("TRN2", target_bir_lowering=False, debug=False,
                   num_devices=N_CORES, num_swdge_queues=N_SWDGE_Q)

    def inp(name, shape, dt=F32):
        return nc.dram_tensor(name, shape, dt, kind="ExternalInput").ap()

    AGG_DT = F32R if USE_F32R else F32
    x_lo = inp("x_lo", [half, D], AGG_DT)
    x_hi = inp("x_hi", [half, D], AGG_DT)
    xt_d = inp("xt", [D + 1, cp])          # x^T with ones row
    gidx1_d = inp("gidx1", [128, J1], I16)
    dstf1_d = inp("dstf1", [128, c1])
    gidx2_d = inp("gidx2", [128, J2], I16)
    dstf2_d = inp("dstf2", [128, c2])
    kf2_d = inp("kf2", [128, c2])
    rc_d = inp("rc", [128, nw])
    rc2_d = inp("rc2", [D, nw * 128])
    iota_d = inp("iota", [128, 128])
    iota32_d = inp("iota32", [128, D])     # floor(j/2) pattern
    ident_d = inp("ident", [128, 128])
    wcb_d = inp("wcb", [D + 1, HID])       # [W1r+Wskip; b1+bskip]
    w1l_d = inp("w1l", [D, HID])
    w2lr_d = inp("w2lr", [128, 2 * 2 * OUT])  # halves of [W2l|W2r] packed
    gamma_d = inp("gamma_bc", [128, HID])
    beta_d = inp("beta_bc", [128, HID])
    b2_d = inp("b2_bc", [128, OUT])
    out_d = nc.dram_tensor("out", [cp, OUT], F32, kind="ExternalOutput").ap()

    with tile.TileContext(nc) as tc:
        with (
            tc.tile_pool(name="res", bufs=1) as res,
            tc.tile_pool(name="dram", bufs=1, space="DRAM") as dram,
        ):
            # ---- resident tiles / constants
            gidx1_sb = res.tile([128, J1], I16)
            nc.sync.dma_start(out=gidx1_sb[:], in_=gidx1_d[:])
            dstf1_sb = res.tile([128, c1], F32)
            nc.sync.dma_start(out=dstf1_sb[:], in_=dstf1_d[:])
            gidx2_sb = res.tile([128, J2], I16)
            nc.sync.dma_start(out=gidx2_sb[:], in_=gidx2_d[:])
            dstf2_sb = res.tile([128, c2], F32)
            nc.sync.dma_start(out=dstf2_sb[:], in_=dstf2_d[:])
            kf2_sb = res.tile([128, c2], F32)
            nc.sync.dma_start(out=kf2_sb[:], in_=kf2_d[:])
            rc_sb = res.tile([128, nw], F32)
            nc.sync.dma_start(out=rc_sb[:], in_=rc_d[:])
            rc2_sb = res.tile([D, nw, 128], F32)
            nc.sync.dma_start(out=rc2_sb[:], in_=rc2_d[:])
            iota_sb = res.tile([128, 128], F32)
            nc.sync.dma_start(out=iota_sb[:], in_=iota_d[:])
            iota32_sb = res.tile([128, D], F32)
            nc.sync.dma_start(out=iota32_sb[:], in_=iota32_d[:])
            ident_sb = res.tile([128, 128], F32)
            nc.sync.dma_start(out=ident_sb[:], in_=ident_d[:])
            xt_sb = res.tile([D + 1, cp], F32)
            nc.sync.dma_start(out=xt_sb[:], in_=xt_d[:])
            wcb_sb = res.tile([D + 1, HID], F32)
            nc.sync.dma_start(out=wcb_sb[:], in_=wcb_d[:])
            w1l_sb = res.tile([D, HID], F32)
            nc.sync.dma_start(out=w1l_sb[:], in_=w1l_d[:])
            w2lr_sb = res.tile([128, 2 * 2 * OUT], F32)
            nc.sync.dma_start(out=w2lr_sb[:], in_=w2lr_d[:])
            gamma_sb = res.tile([128, HID], F32)
            nc.sync.dma_start(out=gamma_sb[:], in_=gamma_d[:])
            beta_sb = res.tile([128, HID], F32)
            nc.sync.dma_start(out=beta_sb[:], in_=beta_d[:])
            b2_sb = res.tile([128, OUT], F32)
            nc.sync.dma_start(out=b2_sb[:], in_=b2_d[:])

            agg = res.tile([D, nw, 128], F32)
            nc.vector.memset(agg[:], 0.0)
            pr_sb = res.tile([128, nw, 2 * OUT], F32)
            out_sb = res.tile([128, nw, OUT], F32)

            pk_local = dram.tile([cp, OUT], F32)
            pk_all = dram.tile([npad // 32, D], F32)

            # =========== conv1 aggregation + interleaved dense ===========
            sched1 = p1["sched"]
            with (
                tc.tile_pool(name="gpool", bufs=6) as gpool,
                tc.tile_pool(name="opool", bufs=16) as opool,
                tc.tile_pool(name="pwpool", bufs=3, space="PSUM") as pwp,
                tc.tile_pool(name="dwork", bufs=3) as dwork,
                tc.tile_pool(name="dsmall", bufs=4) as dsmall,
                tc.tile_pool(name="dpsum", bufs=1, space="PSUM") as dpsum,
                tc.tile_pool(name="dpsum2", bufs=2, space="PSUM") as dpsum2,
            ):
                # issue all conv1 gathers (both streams), batched
                gbufs = {}
                for s, table in ((0, x_lo), (1, x_hi)):
                    rows = [r for r in sched1[s] if r[1] >= 0]
                    if not rows:
                        continue
                    c0 = rows[0][1]
                    cend = rows[-1][2] + 1
                    for b0 in range(c0, cend, BATCH_CHUNKS):
                        b1 = min(b0 + BATCH_CHUNKS, cend)
                        g = gpool.tile([128, BATCH_CHUNKS, D], AGG_DT,
                                       tag="gbuf")
                        n_idx = (b1 - b0) * 128
                        nc.gpsimd.dma_gather(
                            out_ap=g[:, 0 : b1 - b0, :],
                            in_ap=table,
                            idxs_ap=gidx1_sb[:, b0 * 8 : b1 * 8],
                            num_idxs=n_idx,
                            num_idxs_reg=n_idx,
                            elem_size=D,
                            single_packet=False,
                            queue_num=len(gbufs) % N_SWDGE_Q,
                        )
                        gbufs[s, b0] = g

                def chunks_psum(first, last, dstf_sb, gb_of):
                    """one-hot matmuls for chunks [first..last] into a PSUM
                    tile (no epilogue — callers defer the agg add)."""
                    pw = pwp.tile([D, 128], F32, tag="pw")
                    for g in range(first, last + 1):
                        gb, col = gb_of(g)
                        o = opool.tile([128, 128], AGG_DT, tag="O")
                        nc.vector.tensor_scalar(
                            out=o[:], in0=iota_sb[:],
                            scalar1=dstf_sb[:, g : g + 1], scalar2=None,
                            op0=mybir.AluOpType.is_equal,
                        )
                        nc.tensor.matmul(
                            pw[:], gb[:, col, :], o[:],
                            start=(g == first), stop=(g == last),
                        )
                    return pw

                def agg_add(w, pw):
                    nc.vector.tensor_tensor(
                        out=agg[:, w, :], in0=agg[:, w, :], in1=pw[:],
                        op=mybir.AluOpType.add,
                    )

                def gb_of1(s):
                    rows = [r for r in sched1[s] if r[1] >= 0]
                    c0 = rows[0][1]

                    def f(g):
                        b0 = c0 + ((g - c0) // BATCH_CHUNKS) * BATCH_CHUNKS
                        return gbufs[s, b0], g - b0
                    return f

                # stream 0: accumulation, epilogue deferred one window
                if any(r[1] >= 0 for r in sched1[0]):
                    f0 = gb_of1(0)
                    pend = []
                    for w, first, last in sched1[0]:
                        if first > last:
                            continue
                        pw = chunks_psum(first, last, dstf1_sb, f0)
                        if len(pend) == 2:
                            agg_add(*pend.pop(0))
                        pend.append((w, pw))
                    for p_ in pend:
                        agg_add(*p_)

                # stream 1: accumulate; agg add + dense deferred one window
                f1 = gb_of1(1) if any(r[1] >= 0 for r in sched1[1]) else None

                def stream1_epilogue(w, pw):
                    if pw is not None:
                        agg_add(w, pw)
                    dense(w)

                def dense(w):
                    # ---------- dense phase for window w ----------
                    meant = dwork.tile([D, 128], F32, tag="meant")
                    nc.vector.tensor_tensor(
                        out=meant[:], in0=agg[:, w, :], in1=rc2_sb[:, w, :],
                        op=mybir.AluOpType.mult,
                    )

                    # x1 = x@Wc + bc + mean@W1l   [128, HID]
                    x1p = dpsum2.tile([128, HID], F32, tag="x1")
                    nc.tensor.matmul(
                        x1p[:], xt_sb[:, 128 * w : 128 * (w + 1)], wcb_sb[:],
                        start=True, stop=False)
                    nc.tensor.matmul(x1p[:], meant[:], w1l_sb[:],
                                     start=False, stop=True)

                    # LayerNorm + ELU
                    mu = dsmall.tile([128, 1], F32, tag="mu")
                    nc.vector.reduce_sum(out=mu[:], in_=x1p[:],
                                         axis=mybir.AxisListType.X)
                    nc.vector.tensor_scalar(
                        out=mu[:], in0=mu[:], scalar1=1.0 / HID,
                        scalar2=None, op0=mybir.AluOpType.mult)
                    xc = dwork.tile([128, HID], F32, tag="xc")
                    nc.vector.tensor_scalar(
                        out=xc[:], in0=x1p[:], scalar1=mu[:], scalar2=None,
                        op0=mybir.AluOpType.subtract)
                    sq = dwork.tile([128, HID], F32, tag="sq")
                    var = dsmall.tile([128, 1], F32, tag="var")
                    nc.vector.scalar_tensor_tensor(
                        out=sq[:], in0=xc[:], scalar=1.0, in1=xc[:],
                        op0=mybir.AluOpType.mult, op1=mybir.AluOpType.mult,
                        accum_out=var[:])
                    rstd = dsmall.tile([128, 1], F32, tag="rstd")
                    nc.vector.tensor_scalar(
                        out=rstd[:], in0=var[:], scalar1=1.0 / HID,
                        scalar2=LN_EPS, op0=mybir.AluOpType.mult,
                        op1=mybir.AluOpType.add)
                    nc.scalar.activation(
                        rstd[:], rstd[:], mybir.ActivationFunctionType.Sqrt)
                    nc.vector.reciprocal(rstd[:], rstd[:])
                    y = dwork.tile([128, HID], F32, tag="y")
                    nc.vector.scalar_tensor_tensor(
                        out=y[:], in0=xc[:], scalar=rstd[:], in1=gamma_sb[:],
                        op0=mybir.AluOpType.mult, op1=mybir.AluOpType.mult)
                    nc.vector.tensor_tensor(
                        out=y[:], in0=y[:], in1=beta_sb[:],
                        op=mybir.AluOpType.add)
                    # ELU: h = (max(y,0)-1) + exp(min(y,0))
                    m0 = dwork.tile([128, HID], F32, tag="m0")
                    nc.vector.tensor_scalar(
                        out=m0[:], in0=y[:], scalar1=0.0, scalar2=None,
                        op0=mybir.AluOpType.min)
                    ex = dwork.tile([128, HID], F32, tag="ex")
                    nc.scalar.activation(
                        ex[:], m0[:], mybir.ActivationFunctionType.Exp)
                    rm1 = dwork.tile([128, HID], F32, tag="rm1")
                    nc.vector.tensor_scalar(
                        out=rm1[:], in0=y[:], scalar1=0.0, scalar2=-1.0,
                        op0=mybir.AluOpType.max, op1=mybir.AluOpType.add)
                    h = dwork.tile([128, HID], F32, tag="h")
                    nc.vector.tensor_tensor(
                        out=h[:], in0=rm1[:], in1=ex[:],
                        op=mybir.AluOpType.add)

                    # p | r = h @ [W2l | W2r]
                    prp = dpsum2.tile([128, 2 * OUT], F32, tag="pr")
                    for hh in range(2):
                        tph = dpsum.tile([128, 128], F32, tag="tph")
                        nc.tensor.transpose(
                            tph[:], h[:, 128 * hh : 128 * (hh + 1)],
                            ident_sb[:])
                        hts = dwork.tile([128, 128], F32, tag="hts")
                        nc.scalar.activation(
                            hts[:], tph[:], mybir.ActivationFunctionType.Copy)
                        nc.tensor.matmul(
                            prp[:], hts[:],
                            w2lr_sb[:, 4 * hh : 4 * (hh + 1)],
                            start=(hh == 0), stop=(hh == 1))
                    nc.scalar.activation(
                        pr_sb[:, w, :], prp[:],
                        mybir.ActivationFunctionType.Copy)
                    nc.sync.dma_start(
                        out=pk_local[128 * w : 128 * (w + 1), :],
                        in_=pr_sb[:, w, 0:OUT])

                pend1 = []
                for w, first, last in sched1[1]:
                    pw = (chunks_psum(first, last, dstf1_sb, f1)
                          if f1 is not None and first <= last else None)
                    if len(pend1) == 2:
                        stream1_epilogue(*pend1.pop(0))
                    pend1.append((w, pw))
                for p_ in pend1:
                    stream1_epilogue(*p_)

            # ================= p all-gather (50KB per core) =================
            nc.gpsimd.collective_compute(
                "AllGather",
                mybir.AluOpType.bypass,
                replica_groups=[list(range(N_CORES))],
                ins=[pk_local.opt()],
                outs=[pk_all.opt()],
            )

            # ================= conv2 aggregation =================
            sched2 = p2["sched"][0]
            with (
                tc.tile_pool(name="g2pool", bufs=6) as g2pool,
                tc.tile_pool(name="o2pool", bufs=16) as o2pool,
                tc.tile_pool(name="m2pool", bufs=16) as m2pool,
                tc.tile_pool(name="pw2pool", bufs=3, space="PSUM") as pw2p,
                tc.tile_pool(name="fwork", bufs=8) as fwork,
            ):
                rows = [r for r in sched2 if r[1] >= 0]
                c0 = rows[0][1]
                cend = rows[-1][2] + 1
                g2bufs = {}
                for b0 in range(c0, cend, BATCH_CHUNKS):
                    b1 = min(b0 + BATCH_CHUNKS, cend)
                    g = g2pool.tile([128, BATCH_CHUNKS, D], AGG_DT,
                                    tag="g2buf")
                    n_idx = (b1 - b0) * 128
                    nc.gpsimd.dma_gather(
                        out_ap=g[:, 0 : b1 - b0, :],
                        in_ap=pk_all[:].bitcast(AGG_DT) if USE_F32R
                        else pk_all,
                        idxs_ap=gidx2_sb[:, b0 * 8 : b1 * 8],
                        num_idxs=n_idx,
                        num_idxs_reg=n_idx,
                        elem_size=D,
                        single_packet=False,
                        queue_num=len(g2bufs) % N_SWDGE_Q,
                    )
                    g2bufs[b0] = g

                def conv2_epilogue(w, pw2):
                    # tree-reduce the 32 packed slots: [128,64] -> [128,2]
                    s64 = fwork.tile([128, D], F32, tag="s64")
                    nc.scalar.activation(
                        s64[:], pw2[:], mybir.ActivationFunctionType.Copy)
                    t32 = fwork.tile([128, 32], F32, tag="t32")
                    nc.vector.tensor_tensor(
                        out=t32[:], in0=s64[:, 0:32], in1=s64[:, 32:64],
                        op=mybir.AluOpType.add)
                    t16 = fwork.tile([128, 16], F32, tag="t16")
                    nc.vector.tensor_tensor(
                        out=t16[:], in0=t32[:, 0:16], in1=t32[:, 16:32],
                        op=mybir.AluOpType.add)
                    t8 = fwork.tile([128, 8], F32, tag="t8")
                    nc.vector.tensor_tensor(
                        out=t8[:], in0=t16[:, 0:8], in1=t16[:, 8:16],
                        op=mybir.AluOpType.add)
                    t4 = fwork.tile([128, 4], F32, tag="t4")
                    nc.vector.tensor_tensor(
                        out=t4[:], in0=t8[:, 0:4], in1=t8[:, 4:8],
                        op=mybir.AluOpType.add)
                    t2 = fwork.tile([128, 2], F32, tag="t2")
                    nc.vector.tensor_tensor(
                        out=t2[:], in0=t4[:, 0:2], in1=t4[:, 2:4],
                        op=mybir.AluOpType.add)
                    # out = t2 * rc + r + b2
                    t = fwork.tile([128, OUT], F32, tag="fo")
                    nc.vector.tensor_scalar(
                        out=t[:], in0=t2[:], scalar1=rc_sb[:, w : w + 1],
                        scalar2=None, op0=mybir.AluOpType.mult)
                    nc.vector.tensor_tensor(
                        out=t[:], in0=t[:], in1=pr_sb[:, w, OUT : 2 * OUT],
                        op=mybir.AluOpType.add)
                    nc.vector.tensor_tensor(
                        out=out_sb[:, w, :], in0=t[:], in1=b2_sb[:],
                        op=mybir.AluOpType.add)
                    nc.sync.dma_start(
                        out=out_d[128 * w : 128 * (w + 1), :],
                        in_=out_sb[:, w, :])

                pend2 = []
                for w, first, last in sched2:
                    if first > last:
                        nc.vector.tensor_tensor(
                            out=out_sb[:, w, :],
                            in0=pr_sb[:, w, OUT : 2 * OUT], in1=b2_sb[:],
                            op=mybir.AluOpType.add)
                        nc.sync.dma_start(
                            out=out_d[128 * w : 128 * (w + 1), :],
                            in_=out_sb[:, w, :])
                        continue
                    pw2 = pw2p.tile([128, D], F32, tag="pw2")
                    for g in range(first, last + 1):
                        b0 = c0 + ((g - c0) // BATCH_CHUNKS) * BATCH_CHUNKS
                        gb = g2bufs[b0]
                        # masked row: (iota32 == k) * gathered
                        mt = m2pool.tile([128, D], AGG_DT, tag="mt")
                        nc.vector.scalar_tensor_tensor(
                            out=mt[:], in0=iota32_sb[:],
                            scalar=kf2_sb[:, g : g + 1],
                            in1=gb[:, g - b0, :],
                            op0=mybir.AluOpType.is_equal,
                            op1=mybir.AluOpType.mult)
                        o = o2pool.tile([128, 128], AGG_DT, tag="O2")
                        nc.vector.tensor_scalar(
                            out=o[:], in0=iota_sb[:],
                            scalar1=dstf2_sb[:, g : g + 1], scalar2=None,
                            op0=mybir.AluOpType.is_equal,
                        )
                        nc.tensor.matmul(
                            pw2[:], o[:], mt[:],
                            start=(g == first), stop=(g == last),
                        )
                    if len(pend2) == 2:
                        conv2_epilogue(*pend2.pop(0))
                    pend2.append((w, pw2))
                for p_ in pend2:
                    conv2_epilogue(*p_)


    nc.compile()
    return nc


def make_inputs(plan, x, W1l, W1r, b1, Wskip, bskip, gamma, beta, W2l, W2r,
                b2, n_nodes):
    cp, half, npad, nw = plan["cp"], plan["half"], plan["npad"], plan["nw"]
    xp = np.zeros((npad, D), np.float32)
    xp[:n_nodes] = np.asarray(x, np.float32)
    wc = np.asarray(W1r, np.float32) + np.asarray(Wskip, np.float32)
    bc = np.asarray(b1, np.float32) + np.asarray(bskip, np.float32)
    wcb = np.concatenate([wc, bc[None, :]], axis=0)
    w2lr_full = np.concatenate(
        [np.asarray(W2l, np.float32), np.asarray(W2r, np.float32)], axis=1
    )  # [HID, 4]
    w2lr = (
        w2lr_full.reshape(2, 128, 2 * OUT).transpose(1, 0, 2)
        .reshape(128, 2 * 2 * OUT).copy()
    )
    iota = np.tile(np.arange(128, dtype=np.float32)[None, :], (128, 1))
    iota32 = np.tile(
        (np.arange(D, dtype=np.float32) // 2)[None, :], (128, 1))
    ident = np.eye(128, dtype=np.float32)
    gamma_bc = np.tile(np.asarray(gamma, np.float32)[None, :], (128, 1))
    beta_bc = np.tile(np.asarray(beta, np.float32)[None, :], (128, 1))
    b2_bc = np.tile(np.asarray(b2, np.float32)[None, :], (128, 1))

    common = dict(
        x_lo=xp[:half].copy(), x_hi=xp[half:].copy(),
        iota=iota, iota32=iota32, ident=ident,
        wcb=wcb, w1l=np.asarray(W1l, np.float32), w2lr=w2lr,
        gamma_bc=gamma_bc, beta_bc=beta_bc, b2_bc=b2_bc,
    )
    in_maps = []
    for c in range(N_CORES):
        m = dict(common)
        xc_loc = xp[cp * c : cp * (c + 1)]
        xt = np.empty((D + 1, cp), np.float32)
        xt[0:D] = xc_loc.T
        xt[D] = 1.0
        m["xt"] = xt
        m["gidx1"] = plan["p1"]["gidx_tile"][c]
        m["dstf1"] = plan["p1"]["dstf_tile"][c]
        m["gidx2"] = plan["p2"]["gidx_tile"][c]
        m["dstf2"] = plan["p2"]["dstf_tile"][c]
        m["kf2"] = plan["p2"]["extra_tile"][c]
        m["rc"] = plan["rc_tile"][c]
        rcw = plan["rc_tile"][c].transpose(1, 0).reshape(1, -1)  # [1, nw*128]
        m["rc2"] = np.broadcast_to(rcw, (D, rcw.shape[1])).copy()
        in_maps.append(m)
    return in_maps


_CACHE = {}


def _get_compiled(edge_index, n_nodes):
    key = (edge_index.tobytes()[:512], edge_index.shape, n_nodes)
    if key not in _CACHE:
        plan = make_plan(edge_index, n_nodes)
        nc = build_program(plan)
        _CACHE[key] = (plan, nc)
    return _CACHE[key]


def run(inputs, trace=False):
    x = np.asarray(inputs["x"], np.float32)
    edge_index = np.asarray(inputs["edge_index"], np.int32)
    n_nodes = x.shape[0]
    plan, nc = _get_compiled(edge_index, n_nodes)
    in_maps = make_inputs(
        plan, x, inputs["W1l"], inputs["W1r"], inputs["b1"], inputs["Wskip"],
        inputs["bskip"], inputs["gamma"], inputs["beta"], inputs["W2l"],
        inputs["W2r"], inputs["b2"], n_nodes)
    res = run_bass_kernel_spmd(
        nc, in_maps, list(range(N_CORES)), trace=trace)
    cp = plan["cp"]
    out = np.empty((n_nodes, OUT), np.float32)
    for c in range(N_CORES):
        lo = cp * c
        hi = min(cp * (c + 1), n_nodes)
        out[lo:hi] = res.results[c]["out"][0 : hi - lo]
    return out, res


def kernel(**inputs) -> np.ndarray:
    out, _ = run(inputs)
    return out



# revision 16
# speedup vs baseline: 1.2314x; 1.2314x over previous
"""GeneSAGE (2-layer GraphSAGE + skip + LayerNorm + ELU) on 8 Trainium2 cores.

v2 design. Edge-parallel by destination range: core c owns nodes
[cp*c, cp*(c+1)), cp=6272. Both convs share one edge bucketing: edges
are grouped by (src-half stream, 64-node dst window) with chunks of 128
edge slots, chunk structure common to all 8 cores (SPMD). Per chunk a
[128, 64] one-hot (batched generation: 16 chunks per DVE instruction
via a broadcast is_equal against a resident iota) is matmul'd against
the gathered per-edge payload, accumulating node-major [64dst, *] PSUM
strips which are added into an SBUF agg. Gathers use SWDGE dma_gather
spread round-robin over 4 queues (desc-gen parallelizes across Q7 core
pairs; this is the dominant cost). Conv1 gathers x rows (256B fp32)
from HBM; conv2 gathers 256B rows of a device-built table where row n
= 32 copies of (p0[n], p1[n]), so the matmul rhs is a free 2-column
slice (p = h @ W2l is computed per window in the conv1 dense phase,
AllGather'd, then expanded/written to HBM). The dense phase
(mean/linear/LayerNorm/ELU/p,r) runs per 128-node window interleaved
under the conv1 edge stream with deferred epilogues; LayerNorm mean /
center / variance and PSUM moves run on the Scalar engine.
"""

import numpy as np

import concourse.mybir as mybir
from concourse import bacc, bass, tile
from concourse.bass_utils import run_bass_kernel_spmd

F32 = mybir.dt.float32
I16 = mybir.dt.int16

N_CORES = 8
D = 64
HID = 256
OUT = 2
LN_EPS = 1e-5
B = 32            # chunks per dma_gather
G = 16            # chunks per one-hot DVE instruction
N_SWDGE_Q = 4
DEBUG = False
AF = mybir.ActivationFunctionType
OP = mybir.AluOpType


def make_plan(edge_index: np.ndarray, n_nodes: int):
    cp = int(np.ceil(n_nodes / (N_CORES * 128))) * 128
    nw = cp // 128
    nw64 = cp // 64
    npad = N_CORES * cp
    half = npad // 2
    assert half <= 32768

    src = edge_index[0].astype(np.int64)
    dst = edge_index[1].astype(np.int64)
    E = src.shape[0]

    deg = np.bincount(dst, minlength=npad).astype(np.float64)
    rc = (1.0 / np.maximum(deg, 1.0)).astype(np.float32)
    rc_tile = rc.reshape(N_CORES, nw, 128).transpose(0, 2, 1).copy()

    core = dst // cp
    win64 = (dst % cp) // 64
    stream = (src >= half).astype(np.int64)
    ngrp = 2 * nw64
    g = stream * nw64 + win64
    key = core * ngrp + g
    order = np.argsort(key, kind="stable")
    counts = np.bincount(key, minlength=N_CORES * ngrp).reshape(
        N_CORES, ngrp)
    nch = -(-counts.max(axis=0) // 128)          # [ngrp] common structure
    off = np.zeros(ngrp, np.int64)
    running = 0
    for gg in range(ngrp):
        off[gg] = running
        running += nch[gg]
    c_total = int(running)
    e_slots = c_total * 128

    sk = key[order]
    grp_start = np.searchsorted(sk, np.arange(N_CORES * ngrp))
    rank = np.arange(E) - grp_start[sk]
    g_of = sk % ngrp
    c_of = sk // ngrp
    slot = off[g_of] * 128 + rank

    gidx = np.zeros((N_CORES, e_slots), np.int16)
    dstf = np.full((N_CORES, e_slots), -1.0, np.float32)
    gidx[c_of, slot] = (src[order] % half).astype(np.int16)
    dstf[c_of, slot] = (dst[order] % 64).astype(np.float32)

    a = gidx.reshape(N_CORES, e_slots // 16, 16).transpose(0, 2, 1)
    gidx_tile = np.tile(a, (1, 8, 1)).copy()      # [c, 128, J]
    dstf_tile = dstf.reshape(N_CORES, c_total, 128).transpose(0, 2, 1).copy()

    # stream chunk ranges (stream-major group order => contiguous)
    s1_start = int(off[nw64]) if nch[:nw64].sum() else 0
    stream_ranges = [(0, s1_start), (s1_start, c_total)]

    # per-chunk info: (w, s2, g_start, g_stop, s_start, s_stop).
    # conv1 accumulates per group (contiguous chunks) and adds into an
    # SBUF agg; conv2 accumulates per strip in resident PSUM, so it uses
    # strip-level start/stop flags that span both streams.
    chunk_info = [None] * c_total
    last_chunk = np.full(nw, -1, np.int64)
    for w in range(nw):
        for s2 in range(2):
            present = [s * nw64 + 2 * w + s2 for s in range(2)
                       if nch[s * nw64 + 2 * w + s2] > 0]
            if not present:
                continue
            first = int(off[present[0]])
            last = int(off[present[-1]] + nch[present[-1]] - 1)
            for gg in present:
                gfirst = int(off[gg])
                glast = int(off[gg] + nch[gg] - 1)
                for c in range(gfirst, glast + 1):
                    chunk_info[c] = (w, s2, c == gfirst, c == glast,
                                     c == first, c == last)
            last_chunk[w] = max(last_chunk[w], last)
    fire = {}
    for w in range(nw):
        if last_chunk[w] >= 0:
            fire.setdefault(int(last_chunk[w]), []).append(w)

    return dict(cp=cp, nw=nw, nw64=nw64, npad=npad, half=half,
                c_total=c_total, gidx_tile=gidx_tile, dstf_tile=dstf_tile,
                rc_tile=rc_tile, stream_ranges=stream_ranges,
                chunk_info=chunk_info, fire=fire)


def build_program(plan):
    cp, nw, npad, half = plan["cp"], plan["nw"], plan["npad"], plan["half"]
    c_total = plan["c_total"]
    J = c_total * 8
    nblk = npad // 128          # 392 rows per partition in pk/tab layouts
    chunk_info = plan["chunk_info"]
    fire = plan["fire"]
    stream_ranges = plan["stream_ranges"]

    nc = bacc.Bacc("TRN2", target_bir_lowering=False, debug=False,
                   num_devices=N_CORES, num_swdge_queues=N_SWDGE_Q)

    def inp(name, shape, dt=F32):
        return nc.dram_tensor(name, shape, dt, kind="ExternalInput").ap()

    x_lo = inp("x_lo", [half, D])
    x_hi = inp("x_hi", [half, D])
    xt_d = inp("xt", [D + 1, cp])
    gidx_d = inp("gidx", [128, J], I16)
    dstf_d = inp("dstf", [128, c_total])
    iota_d = inp("iota64", [128, G, 64])
    ident_d = inp("ident", [128, 128])
    wcb_d = inp("wcb", [D + 1, HID])
    w1l_d = inp("w1l", [D, HID])
    w2lr_d = inp("w2lr", [128, 2 * 2 * OUT])
    gamma_d = inp("gamma_bc", [128, HID])
    beta_d = inp("beta_bc", [128, HID])
    b2_d = inp("b2_bc", [128, OUT])
    rc_d = inp("rc", [128, nw])
    out_d = nc.dram_tensor("out", [cp, OUT], F32, kind="ExternalOutput").ap()
    if DEBUG:
        dagg_d = nc.dram_tensor("dagg", [cp, D], F32,
                                kind="ExternalOutput").ap()
        dpr_d = nc.dram_tensor("dpr", [cp, 2 * OUT], F32,
                               kind="ExternalOutput").ap()
        dpa_d = nc.dram_tensor("dpa", [128, (npad // 128) * OUT], F32,
                               kind="ExternalOutput").ap()
        dagg2_d = nc.dram_tensor("dagg2", [cp, OUT], F32,
                                 kind="ExternalOutput").ap()

    with tile.TileContext(nc) as tc:
        with (
            tc.tile_pool(name="res", bufs=1) as res,
            tc.tile_pool(name="dram", bufs=1, space="DRAM") as dram,
        ):
            def load(name, shape, src, dt=F32):
                t = res.tile(shape, dt, tag=name)
                nc.sync.dma_start(out=t[:], in_=src[:])
                return t

            gidx_sb = load("gidx", [128, J], gidx_d, I16)
            dstf_sb = load("dstf", [128, c_total], dstf_d)
            iota_sb = load("iota", [128, G, 64], iota_d)
            ident_sb = load("ident", [128, 128], ident_d)
            xt_sb = load("xt", [D + 1, cp], xt_d)
            wcb_sb = load("wcb", [D + 1, HID], wcb_d)
            w1l_sb = load("w1l", [D, HID], w1l_d)
            w2lr_sb = load("w2lr", [128, 2 * 2 * OUT], w2lr_d)
            gamma_sb = load("gamma", [128, HID], gamma_d)
            beta_sb = load("beta", [128, HID], beta_d)
            b2_sb = load("b2", [128, OUT], b2_d)
            rc_sb = load("rc", [128, nw], rc_d)

            agg = res.tile([128, nw, D], F32, tag="agg")
            nc.vector.memset(agg[:], 0.0)
            pr_sb = res.tile([128, nw, 2 * OUT], F32, tag="prs")
            out_sb = res.tile([128, nw, OUT], F32, tag="outs")
            pa_sb = res.tile([128, nblk, OUT], F32, tag="pas")

            pk_local = dram.tile([cp, OUT], F32)
            pk_all = dram.tile([128, nblk, OUT], F32)
            tab2 = dram.tile([128, nblk, D], F32)
            tab2_flat = tab2[:].rearrange("q s d -> (q s) d")

            qi = [0]

            def emit_gathers(tables, gpool, tag):
                gmap = {}
                for s in range(2):
                    c0, cend = stream_ranges[s]
                    for b0 in range(c0, cend, B):
                        b1 = min(b0 + B, cend)
                        g_t = gpool.tile([128, B, D], F32, tag=tag)
                        n_idx = (b1 - b0) * 128
                        nc.gpsimd.dma_gather(
                            out_ap=g_t[:, 0:b1 - b0, :],
                            in_ap=tables[s],
                            idxs_ap=gidx_sb[:, b0 * 8:b1 * 8],
                            num_idxs=n_idx,
                            num_idxs_reg=n_idx,
                            elem_size=D,
                            single_packet=False,
                            queue_num=qi[0] % N_SWDGE_Q,
                        )
                        qi[0] += 1
                        for c in range(b0, b1):
                            gmap[c] = (g_t, c - b0)
                return gmap

            def onehot_for(c, opool, obufs):
                cb = (c // G) * G
                if cb not in obufs:
                    o_t = opool.tile([128, G, 64], F32, tag="o")
                    n = min(G, c_total - cb)
                    src3 = dstf_sb[:, cb:cb + n].unsqueeze(2).broadcast_to(
                        [128, n, 64])
                    nc.vector.tensor_tensor(
                        out=o_t[:, 0:n, :], in0=iota_sb[:, 0:n, :],
                        in1=src3, op=OP.is_equal)
                    obufs.clear()
                    obufs[cb] = o_t
                return obufs[cb]

            # =================== conv1 ===================
            with (
                tc.tile_pool(name="gpool", bufs=6) as gpool,
                tc.tile_pool(name="opool", bufs=4) as opool,
                tc.tile_pool(name="pwp", bufs=3, space="PSUM") as pwp,
                tc.tile_pool(name="dps1", bufs=1, space="PSUM") as dps1,
                tc.tile_pool(name="dps2", bufs=1, space="PSUM") as dps2,
                tc.tile_pool(name="dwork", bufs=2) as dwork,
                tc.tile_pool(name="mwork", bufs=4) as mwork,
                tc.tile_pool(name="dsmall", bufs=4) as dsmall,
            ):
                gmap = emit_gathers((x_lo, x_hi), gpool, "g1")

                def dense(w):
                    mean_sb = mwork.tile([128, D], F32, tag="mean")
                    nc.vector.tensor_scalar(
                        out=mean_sb[:], in0=agg[:, w, :],
                        scalar1=rc_sb[:, w:w + 1], scalar2=None,
                        op0=OP.mult)
                    tp = dps1.tile([64, 128], F32, tag="tp")
                    nc.tensor.transpose(tp[:], mean_sb[:], ident_sb[:])
                    meant = mwork.tile([64, 128], F32, tag="meant")
                    nc.scalar.activation(meant[:], tp[:], AF.Copy)

                    x1p = dps2.tile([128, HID], F32, tag="x1")
                    nc.tensor.matmul(
                        x1p[:], xt_sb[:, 128 * w:128 * (w + 1)], wcb_sb[:],
                        start=True, stop=False)
                    nc.tensor.matmul(x1p[:], meant[:], w1l_sb[:],
                                     start=False, stop=True)

                    # LayerNorm
                    musum = dsmall.tile([128, 1], F32, tag="mu")
                    nc.vector.reduce_sum(out=musum[:], in_=x1p[:],
                                         axis=mybir.AxisListType.X)
                    mu = dsmall.tile([128, 1], F32, tag="muv")
                    nc.vector.tensor_scalar(
                        out=mu[:], in0=musum[:], scalar1=1.0 / HID,
                        scalar2=None, op0=OP.mult)
                    xc = dwork.tile([128, HID], F32, tag="xc")
                    nc.vector.tensor_scalar(
                        out=xc[:], in0=x1p[:], scalar1=mu[:], scalar2=None,
                        op0=OP.subtract)
                    sq = dwork.tile([128, HID], F32, tag="sq")
                    var = dsmall.tile([128, 1], F32, tag="var")
                    nc.vector.scalar_tensor_tensor(
                        out=sq[:], in0=xc[:], scalar=1.0, in1=xc[:],
                        op0=OP.mult, op1=OP.mult, accum_out=var[:])
                    rstd = dsmall.tile([128, 1], F32, tag="rstd")
                    nc.vector.tensor_scalar(
                        out=rstd[:], in0=var[:], scalar1=1.0 / HID,
                        scalar2=LN_EPS, op0=OP.mult, op1=OP.add)
                    nc.scalar.activation(rstd[:], rstd[:], AF.Sqrt)
                    nc.vector.reciprocal(rstd[:], rstd[:])
                    y = dwork.tile([128, HID], F32, tag="y")
                    nc.vector.scalar_tensor_tensor(
                        out=y[:], in0=xc[:], scalar=rstd[:], in1=gamma_sb[:],
                        op0=OP.mult, op1=OP.mult)
                    nc.vector.tensor_tensor(
                        out=y[:], in0=y[:], in1=beta_sb[:], op=OP.add)
                    # ELU = (max(y,0)-1) + exp(min(y,0))
                    m0 = dwork.tile([128, HID], F32, tag="m0")
                    nc.vector.tensor_scalar(
                        out=m0[:], in0=y[:], scalar1=0.0, scalar2=None,
                        op0=OP.min)
                    ex = dwork.tile([128, HID], F32, tag="ex")
                    nc.scalar.activation(ex[:], m0[:], AF.Exp)
                    rm1 = dwork.tile([128, HID], F32, tag="rm1")
                    nc.vector.tensor_scalar(
                        out=rm1[:], in0=y[:], scalar1=0.0, scalar2=-1.0,
                        op0=OP.max, op1=OP.add)
                    h = dwork.tile([128, HID], F32, tag="h")
                    nc.vector.tensor_tensor(
                        out=h[:], in0=rm1[:], in1=ex[:], op=OP.add)

                    # p | r = h @ [W2l | W2r]
                    prp = dps2.tile([128, 2 * OUT], F32, tag="pr")
                    for hh in range(2):
                        tph = dps1.tile([128, 128], F32, tag="tph")
                        nc.tensor.transpose(
                            tph[:], h[:, 128 * hh:128 * (hh + 1)],
                            ident_sb[:])
                        hts = mwork.tile([128, 128], F32, tag="hts")
                        nc.scalar.activation(hts[:], tph[:], AF.Copy)
                        nc.tensor.matmul(
                            prp[:], hts[:], w2lr_sb[:, 4 * hh:4 * (hh + 1)],
                            start=(hh == 0), stop=(hh == 1))
                    nc.scalar.activation(pr_sb[:, w, :], prp[:], AF.Copy)
                    nc.sync.dma_start(
                        out=pk_local[128 * w:128 * (w + 1), :],
                        in_=pr_sb[:, w, 0:OUT])
                    if DEBUG:
                        nc.sync.dma_start(
                            out=dagg_d[128 * w:128 * (w + 1), :],
                            in_=agg[:, w, :])
                        nc.sync.dma_start(
                            out=dpr_d[128 * w:128 * (w + 1), :],
                            in_=pr_sb[:, w, :])

                obufs = {}
                pend_add = []
                pend_dense = []
                pw = None
                for c in range(c_total):
                    info = chunk_info[c]
                    if info is None:
                        continue
                    w, s2, g_start, g_stop, _, _ = info
                    o_t = onehot_for(c, opool, obufs)
                    g_t, col = gmap[c]
                    if g_start:
                        pw = pwp.tile([64, D], F32, tag="pw")
                    nc.tensor.matmul(
                        pw[:], o_t[:, c - (c // G) * G, :], g_t[:, col, :],
                        start=g_start, stop=g_stop)
                    if g_stop:
                        pend_add.append((w, s2, pw))
                        if len(pend_add) == 3:
                            aw, as2, apw = pend_add.pop(0)
                            nc.vector.tensor_tensor(
                                out=agg[64 * as2:64 * as2 + 64, aw, :],
                                in0=agg[64 * as2:64 * as2 + 64, aw, :],
                                in1=apw[:], op=OP.add)
                    if c in fire:
                        for wf in fire[c]:
                            # flush any pending adds for this window
                            keep = []
                            for item in pend_add:
                                if item[0] == wf:
                                    aw, as2, apw = item
                                    nc.vector.tensor_tensor(
                                        out=agg[64 * as2:64 * as2 + 64,
                                                aw, :],
                                        in0=agg[64 * as2:64 * as2 + 64,
                                                aw, :],
                                        in1=apw[:], op=OP.add)
                                else:
                                    keep.append(item)
                            pend_add = keep
                            pend_dense.append(wf)
                            if len(pend_dense) == 3:
                                dense(pend_dense.pop(0))
                for aw, as2, apw in pend_add:
                    nc.vector.tensor_tensor(
                        out=agg[64 * as2:64 * as2 + 64, aw, :],
                        in0=agg[64 * as2:64 * as2 + 64, aw, :],
                        in1=apw[:], op=OP.add)
                for wf in pend_dense:
                    dense(wf)

            # =============== AllGather p + build conv2 table ===============
            nc.gpsimd.collective_compute(
                "AllGather",
                OP.bypass,
                replica_groups=[list(range(N_CORES))],
                ins=[pk_local.opt()],
                outs=[pk_all.opt()],
            )
            nc.sync.dma_start(out=pa_sb[:], in_=pk_all[:])
            if DEBUG:
                nc.sync.dma_start(out=dpa_d[:], in_=pa_sb[:])
            with tc.tile_pool(name="tabp", bufs=2) as tabp:
                npiece = 4
                pb = nblk // npiece
                for pc in range(npiece):
                    tp_ = tabp.tile([128, pb, 32, OUT], F32, tag="tp")
                    srcv = pa_sb[:, pb * pc:pb * (pc + 1), :].unsqueeze(
                        2).broadcast_to([128, pb, 32, OUT])
                    nc.vector.tensor_copy(out=tp_[:], in_=srcv)
                    nc.sync.dma_start(
                        out=tab2[:, pb * pc:pb * (pc + 1), :], in_=tp_[:])

            # =================== conv2 ===================
            with (
                tc.tile_pool(name="gpool2", bufs=6) as gpool2,
                tc.tile_pool(name="opool2", bufs=4) as opool2,
                tc.tile_pool(name="pw2p", bufs=3, space="PSUM") as pw2p,
                tc.tile_pool(name="ewk", bufs=6) as ewk,
            ):
                agg2 = res.tile([128, nw, OUT], F32, tag="agg2")
                nc.vector.memset(agg2[:], 0.0)
                gmap2 = emit_gathers(
                    (tab2_flat[0:half, :], tab2_flat[half:npad, :]),
                    gpool2, "g2")

                def epi2(w):
                    if DEBUG:
                        nc.sync.dma_start(
                            out=dagg2_d[128 * w:128 * (w + 1), :],
                            in_=agg2[:, w, :])
                    t = ewk.tile([128, OUT], F32, tag="t")
                    nc.vector.tensor_scalar(
                        out=t[:], in0=agg2[:, w, :],
                        scalar1=rc_sb[:, w:w + 1], scalar2=None,
                        op0=OP.mult)
                    nc.vector.tensor_tensor(
                        out=t[:], in0=t[:], in1=pr_sb[:, w, OUT:2 * OUT],
                        op=OP.add)
                    nc.vector.tensor_tensor(
                        out=out_sb[:, w, :], in0=t[:], in1=b2_sb[:],
                        op=OP.add)
                    nc.sync.dma_start(
                        out=out_d[128 * w:128 * (w + 1), :],
                        in_=out_sb[:, w, :])

                obufs2 = {}
                pend_add2 = []
                pend2 = []
                pw2 = None
                for c in range(c_total):
                    info = chunk_info[c]
                    if info is None:
                        continue
                    w, s2, g_start, g_stop, _, _ = info
                    o_t = onehot_for(c, opool2, obufs2)
                    g_t, col = gmap2[c]
                    if g_start:
                        pw2 = pw2p.tile([64, OUT], F32, tag="pw2")
                    nc.tensor.matmul(
                        pw2[:], o_t[:, c - (c // G) * G, :],
                        g_t[:, col, 0:OUT],
                        start=g_start, stop=g_stop)
                    if g_stop:
                        pend_add2.append((w, s2, pw2))
                        if len(pend_add2) == 3:
                            aw, as2, apw = pend_add2.pop(0)
                            nc.vector.tensor_tensor(
                                out=agg2[64 * as2:64 * as2 + 64, aw, :],
                                in0=agg2[64 * as2:64 * as2 + 64, aw, :],
                                in1=apw[:], op=OP.add)
                    if c in fire:
                        for wf in fire[c]:
                            keep = []
                            for item in pend_add2:
                                if item[0] == wf:
                                    aw, as2, apw = item
                                    nc.vector.tensor_tensor(
                                        out=agg2[64 * as2:64 * as2 + 64,
                                                 aw, :],
                                        in0=agg2[64 * as2:64 * as2 + 64,
                                                 aw, :],
                                        in1=apw[:], op=OP.add)
                                else:
                                    keep.append(item)
                            pend_add2 = keep
                            pend2.append(wf)
                            if len(pend2) == 3:
                                epi2(pend2.pop(0))
                for aw, as2, apw in pend_add2:
                    nc.vector.tensor_tensor(
                        out=agg2[64 * as2:64 * as2 + 64, aw, :],
                        in0=agg2[64 * as2:64 * as2 + 64, aw, :],
                        in1=apw[:], op=OP.add)
                for wf in pend2:
                    epi2(wf)

    nc.compile()
    return nc


def make_inputs(plan, x, W1l, W1r, b1, Wskip, bskip, gamma, beta, W2l, W2r,
                b2, n_nodes):
    cp, half, npad, nw = plan["cp"], plan["half"], plan["npad"], plan["nw"]
    xp = np.zeros((npad, D), np.float32)
    xp[:n_nodes] = np.asarray(x, np.float32)
    wc = np.asarray(W1r, np.float32) + np.asarray(Wskip, np.float32)
    bc = np.asarray(b1, np.float32) + np.asarray(bskip, np.float32)
    wcb = np.concatenate([wc, bc[None, :]], axis=0)
    w2lr_full = np.concatenate(
        [np.asarray(W2l, np.float32), np.asarray(W2r, np.float32)], axis=1)
    w2lr = (
        w2lr_full.reshape(2, 128, 2 * OUT).transpose(1, 0, 2)
        .reshape(128, 2 * 2 * OUT).copy()
    )
    iota64 = np.broadcast_to(
        np.arange(64, dtype=np.float32)[None, None, :],
        (128, G, 64)).copy()
    ident = np.eye(128, dtype=np.float32)
    gamma_bc = np.tile(np.asarray(gamma, np.float32)[None, :], (128, 1))
    beta_bc = np.tile(np.asarray(beta, np.float32)[None, :], (128, 1))
    b2_bc = np.tile(np.asarray(b2, np.float32)[None, :], (128, 1))

    common = dict(
        x_lo=xp[:half].copy(), x_hi=xp[half:].copy(),
        iota64=iota64, ident=ident,
        wcb=wcb, w1l=np.asarray(W1l, np.float32), w2lr=w2lr,
        gamma_bc=gamma_bc, beta_bc=beta_bc, b2_bc=b2_bc,
    )
    in_maps = []
    for c in range(N_CORES):
        m = dict(common)
        xc_loc = xp[cp * c:cp * (c + 1)]
        xt = np.empty((D + 1, cp), np.float32)
        xt[0:D] = xc_loc.T
        xt[D] = 1.0
        m["xt"] = xt
        m["gidx"] = plan["gidx_tile"][c]
        m["dstf"] = plan["dstf_tile"][c]
        m["rc"] = plan["rc_tile"][c]
        in_maps.append(m)
    return in_maps


_CACHE = {}


def _get_compiled(edge_index, n_nodes):
    key = (edge_index.tobytes()[:512], edge_index.shape, n_nodes)
    if key not in _CACHE:
        plan = make_plan(edge_index, n_nodes)
        nc = build_program(plan)
        _CACHE[key] = (plan, nc)
    return _CACHE[key]


def run(inputs, trace=False):
    x = np.asarray(inputs["x"], np.float32)
    edge_index = np.asarray(inputs["edge_index"], np.int32)
    n_nodes = x.shape[0]
    plan, nc = _get_compiled(edge_index, n_nodes)
    in_maps = make_inputs(
        plan, x, inputs["W1l"], inputs["W1r"], inputs["b1"], inputs["Wskip"],
        inputs["bskip"], inputs["gamma"], inputs["beta"], inputs["W2l"],
        inputs["W2r"], inputs["b2"], n_nodes)
    res = run_bass_kernel_spmd(
        nc, in_maps, list(range(N_CORES)), trace=trace)
    cp = plan["cp"]
    out = np.empty((n_nodes, OUT), np.float32)
    for c in range(N_CORES):
        lo = cp * c
        hi = min(cp * (c + 1), n_nodes)
        out[lo:hi] = res.results[c]["out"][0:hi - lo]
    return out, res


def kernel(**inputs) -> np.ndarray:
    out, _ = run(inputs)
    return out


# revision 30
# speedup vs baseline: 1.4138x; 1.1481x over previous
"""GeneSAGE (2-layer GraphSAGE + skip + LayerNorm + ELU) on 8 Trainium2 cores.

v2 design. Edge-parallel by destination range: core c owns nodes
[cp*c, cp*(c+1)), cp=6272. Both convs share one edge bucketing: edges
are grouped by (src-half stream, 64-node dst window) with chunks of 128
edge slots, chunk structure common to all 8 cores (SPMD). Per chunk a
[128, 64] one-hot (batched generation: 16 chunks per DVE instruction
via a broadcast is_equal against a resident iota) is matmul'd against
the gathered per-edge payload, accumulating node-major [64dst, *] PSUM
strips which are added into an SBUF agg. Gathers use SWDGE dma_gather
spread round-robin over 4 queues (desc-gen parallelizes across Q7 core
pairs; this is the dominant cost). Conv1 gathers x rows (256B fp32)
from HBM; conv2 gathers 256B rows of a device-built table where row n
= 32 copies of (p0[n], p1[n]), so the matmul rhs is a free 2-column
slice (p = h @ W2l is computed per window in the conv1 dense phase,
AllGather'd, then expanded/written to HBM). The dense phase
(mean/linear/LayerNorm/ELU/p,r) runs per 128-node window interleaved
under the conv1 edge stream with deferred epilogues; LayerNorm mean /
center / variance and PSUM moves run on the Scalar engine.
"""

import numpy as np

import concourse.mybir as mybir
from concourse import bacc, bass, tile
from concourse.bass_utils import run_bass_kernel_spmd

F32 = mybir.dt.float32
I16 = mybir.dt.int16

N_CORES = 8
D = 64
HID = 256
OUT = 2
LN_EPS = 1e-5
B = 16            # chunks per dma_gather
G = 16            # chunks per one-hot DVE instruction
N_SWDGE_Q = 4
DEBUG = False
AF = mybir.ActivationFunctionType
OP = mybir.AluOpType


def make_plan(edge_index: np.ndarray, n_nodes: int):
    cp = int(np.ceil(n_nodes / (N_CORES * 128))) * 128
    nw = cp // 128
    nw64 = cp // 64
    npad = N_CORES * cp
    half = npad // 2
    assert half <= 32768

    src = edge_index[0].astype(np.int64)
    dst = edge_index[1].astype(np.int64)
    E = src.shape[0]

    deg = np.bincount(dst, minlength=npad).astype(np.float64)
    rc = (1.0 / np.maximum(deg, 1.0)).astype(np.float32)
    rc_tile = rc.reshape(N_CORES, nw, 128).transpose(0, 2, 1).copy()

    core = dst // cp
    win64 = (dst % cp) // 64
    parity = (src & 1).astype(np.int64)
    ngrp = 2 * nw64                # window-major groups: (win64, parity)
    g = win64 * 2 + parity
    key = core * ngrp + g
    order = np.argsort(key, kind="stable")
    counts = np.bincount(key, minlength=N_CORES * ngrp).reshape(
        N_CORES, ngrp)
    nch = -(-counts.max(axis=0) // 128)          # [ngrp] common structure
    off = np.zeros(ngrp, np.int64)
    running = 0
    for gg in range(ngrp):
        off[gg] = running
        running += nch[gg]
    c_total = int(running)
    e_slots = c_total * 128

    sk = key[order]
    grp_start = np.searchsorted(sk, np.arange(N_CORES * ngrp))
    rank = np.arange(E) - grp_start[sk]
    g_of = sk % ngrp
    c_of = sk // ngrp
    slot = off[g_of] * 128 + rank

    gidx = np.zeros((N_CORES, e_slots), np.int16)
    dstf = np.full((N_CORES, e_slots), -1.0, np.float32)
    gidx[c_of, slot] = (src[order] >> 1).astype(np.int16)
    dstf[c_of, slot] = (dst[order] % 64).astype(np.float32)

    a = gidx.reshape(N_CORES, e_slots // 16, 16).transpose(0, 2, 1)
    gidx_tile = np.tile(a, (1, 8, 1)).copy()      # [c, 128, J]
    dstf_tile = dstf.reshape(N_CORES, c_total, 128).transpose(0, 2, 1).copy()

    # per-chunk info: (w, s2, par, strip_start, strip_stop). A strip
    # (w, s2) covers both parity groups of win64=2w+s2, which are
    # contiguous in chunk order, so PSUM accumulates per strip directly.
    chunk_info = [None] * c_total
    last_chunk = np.full(nw, -1, np.int64)
    for w in range(nw):
        for s2 in range(2):
            w64 = 2 * w + s2
            present = [w64 * 2 + p for p in range(2) if nch[w64 * 2 + p] > 0]
            if not present:
                continue
            first = int(off[present[0]])
            last = int(off[present[-1]] + nch[present[-1]] - 1)
            for gg in present:
                par = gg % 2
                for c in range(int(off[gg]), int(off[gg] + nch[gg])):
                    chunk_info[c] = (w, s2, par, c == first, c == last)
            last_chunk[w] = max(last_chunk[w], last)
    fire = {}
    for w in range(nw):
        if last_chunk[w] >= 0:
            fire.setdefault(int(last_chunk[w]), []).append(w)

    return dict(cp=cp, nw=nw, nw64=nw64, npad=npad, half=half,
                c_total=c_total, gidx_tile=gidx_tile, dstf_tile=dstf_tile,
                rc_tile=rc_tile, chunk_info=chunk_info, fire=fire)


def build_program(plan):
    cp, nw, npad, half = plan["cp"], plan["nw"], plan["npad"], plan["half"]
    c_total = plan["c_total"]
    J = c_total * 8
    npair = npad // 2
    nblk = npair // 128         # 196 pair-rows per partition
    chunk_info = plan["chunk_info"]
    fire = plan["fire"]

    nc = bacc.Bacc("TRN2", target_bir_lowering=False, debug=False,
                   num_devices=N_CORES, num_swdge_queues=N_SWDGE_Q)

    def inp(name, shape, dt=F32):
        return nc.dram_tensor(name, shape, dt, kind="ExternalInput").ap()

    x2_d = inp("x2", [npair, 2 * D])
    xt_d = inp("xt", [D + 1, cp])
    gidx_d = inp("gidx", [128, J], I16)
    dstf_d = inp("dstf", [128, c_total])
    iota_d = inp("iota64", [128, G, 64])
    ident_d = inp("ident", [128, 128])
    wcb_d = inp("wcb", [D + 1, HID])
    w1l_d = inp("w1l", [D, HID])
    w2lr_d = inp("w2lr", [128, 2 * 2 * OUT])
    gamma_d = inp("gamma_bc", [128, HID])
    beta_d = inp("beta_bc", [128, HID])
    b2_d = inp("b2_bc", [128, OUT])
    rc_d = inp("rc", [128, nw])
    out_d = nc.dram_tensor("out", [cp, OUT], F32, kind="ExternalOutput").ap()
    if DEBUG:
        dagg_d = nc.dram_tensor("dagg", [cp, D], F32,
                                kind="ExternalOutput").ap()
        dpr_d = nc.dram_tensor("dpr", [cp, 2 * OUT], F32,
                               kind="ExternalOutput").ap()
        dpa_d = nc.dram_tensor("dpa", [128, nblk * 2 * OUT], F32,
                               kind="ExternalOutput").ap()

    with tile.TileContext(nc) as tc:
        with (
            tc.tile_pool(name="res", bufs=1) as res,
            tc.tile_pool(name="dram", bufs=1, space="DRAM") as dram,
        ):
            def load(name, shape, src, dt=F32):
                t = res.tile(shape, dt, tag=name)
                nc.sync.dma_start(out=t[:], in_=src[:])
                return t

            gidx_sb = load("gidx", [128, J], gidx_d, I16)
            dstf_sb = load("dstf", [128, c_total], dstf_d)
            iota_sb = load("iota", [128, G, 64], iota_d)
            ident_sb = load("ident", [128, 128], ident_d)
            xt_sb = load("xt", [D + 1, cp], xt_d)
            wcb_sb = load("wcb", [D + 1, HID], wcb_d)
            w1l_sb = load("w1l", [D, HID], w1l_d)
            w2lr_sb = load("w2lr", [128, 2 * 2 * OUT], w2lr_d)
            gamma_sb = load("gamma", [128, HID], gamma_d)
            beta_sb = load("beta", [128, HID], beta_d)
            b2_sb = load("b2", [128, OUT], b2_d)
            rc_sb = load("rc", [128, nw], rc_d)

            pr_sb = res.tile([128, nw, 2 * OUT], F32, tag="prs")
            out_sb = res.tile([128, nw, OUT], F32, tag="outs")
            pa_sb = res.tile([128, nblk, 2 * OUT], F32, tag="pas")

            pk_local = dram.tile([cp, OUT], F32)
            pk_all = dram.tile([128, nblk, 2 * OUT], F32)
            tab2 = dram.tile([128, nblk, D], F32)
            tab2_flat = tab2[:].rearrange("q s d -> (q s) d")

            qi = [0]

            def emit_gathers(table, rowd, gpool, tag):
                gmap = {}
                for b0 in range(0, c_total, B):
                    b1 = min(b0 + B, c_total)
                    g_t = gpool.tile([128, B, rowd], F32, tag=tag)
                    n_idx = (b1 - b0) * 128
                    nc.gpsimd.dma_gather(
                        out_ap=g_t[:, 0:b1 - b0, :],
                        in_ap=table,
                        idxs_ap=gidx_sb[:, b0 * 8:b1 * 8],
                        num_idxs=n_idx,
                        num_idxs_reg=n_idx,
                        elem_size=rowd,
                        single_packet=False,
                        queue_num=qi[0] % N_SWDGE_Q,
                    )
                    qi[0] += 1
                    for c in range(b0, b1):
                        gmap[c] = (g_t, c - b0)
                return gmap

            def onehot_for(c, opool, obufs):
                cb = (c // G) * G
                if cb not in obufs:
                    o_t = opool.tile([128, G, 64], F32, tag="o")
                    n = min(G, c_total - cb)
                    src3 = dstf_sb[:, cb:cb + n].unsqueeze(2).broadcast_to(
                        [128, n, 64])
                    nc.vector.tensor_tensor(
                        out=o_t[:, 0:n, :], in0=iota_sb[:, 0:n, :],
                        in1=src3, op=OP.is_equal)
                    obufs.clear()
                    obufs[cb] = o_t
                return obufs[cb]

            # =================== conv1 ===================
            with (
                tc.tile_pool(name="gpool", bufs=10) as gpool,
                tc.tile_pool(name="opool", bufs=6) as opool,
                tc.tile_pool(name="pwp", bufs=4, space="PSUM") as pwp,
                tc.tile_pool(name="dps1", bufs=1, space="PSUM") as dps1,
                tc.tile_pool(name="dps2", bufs=1, space="PSUM") as dps2,
                tc.tile_pool(name="dwork", bufs=2) as dwork,
                tc.tile_pool(name="mwork", bufs=6) as mwork,
                tc.tile_pool(name="dsmall", bufs=4) as dsmall,
            ):
                gmap = emit_gathers(x2_d, 2 * D, gpool, "g1")
                cur_mean = {}

                def dense(w):
                    mean_sb = cur_mean.pop(w)
                    tp = dps1.tile([64, 128], F32, tag="tp")
                    nc.tensor.transpose(tp[:], mean_sb[:], ident_sb[:])
                    meant = mwork.tile([64, 128], F32, tag="meant")
                    nc.scalar.activation(meant[:], tp[:], AF.Copy)

                    x1p = dps2.tile([128, HID], F32, tag="x1")
                    nc.tensor.matmul(
                        x1p[:], xt_sb[:, 128 * w:128 * (w + 1)], wcb_sb[:],
                        start=True, stop=False)
                    nc.tensor.matmul(x1p[:], meant[:], w1l_sb[:],
                                     start=False, stop=True)

                    # LayerNorm
                    musum = dsmall.tile([128, 1], F32, tag="mu")
                    nc.vector.reduce_sum(out=musum[:], in_=x1p[:],
                                         axis=mybir.AxisListType.X)
                    mu = dsmall.tile([128, 1], F32, tag="muv")
                    nc.vector.tensor_scalar(
                        out=mu[:], in0=musum[:], scalar1=1.0 / HID,
                        scalar2=None, op0=OP.mult)
                    xc = dwork.tile([128, HID], F32, tag="xc")
                    nc.vector.tensor_scalar(
                        out=xc[:], in0=x1p[:], scalar1=mu[:], scalar2=None,
                        op0=OP.subtract)
                    sq = dwork.tile([128, HID], F32, tag="sq")
                    var = dsmall.tile([128, 1], F32, tag="var")
                    nc.vector.scalar_tensor_tensor(
                        out=sq[:], in0=xc[:], scalar=1.0, in1=xc[:],
                        op0=OP.mult, op1=OP.mult, accum_out=var[:])
                    rstd = dsmall.tile([128, 1], F32, tag="rstd")
                    nc.vector.tensor_scalar(
                        out=rstd[:], in0=var[:], scalar1=1.0 / HID,
                        scalar2=LN_EPS, op0=OP.mult, op1=OP.add)
                    nc.scalar.activation(rstd[:], rstd[:], AF.Sqrt)
                    nc.vector.reciprocal(rstd[:], rstd[:])
                    y = dwork.tile([128, HID], F32, tag="y")
                    nc.vector.scalar_tensor_tensor(
                        out=y[:], in0=xc[:], scalar=rstd[:], in1=gamma_sb[:],
                        op0=OP.mult, op1=OP.mult)
                    nc.vector.tensor_tensor(
                        out=y[:], in0=y[:], in1=beta_sb[:], op=OP.add)
                    # ELU = (max(y,0)-1) + exp(min(y,0))
                    m0 = dwork.tile([128, HID], F32, tag="m0")
                    nc.vector.tensor_scalar(
                        out=m0[:], in0=y[:], scalar1=0.0, scalar2=None,
                        op0=OP.min)
                    ex = dwork.tile([128, HID], F32, tag="ex")
                    nc.scalar.activation(ex[:], m0[:], AF.Exp)
                    rm1 = dwork.tile([128, HID], F32, tag="rm1")
                    nc.vector.tensor_scalar(
                        out=rm1[:], in0=y[:], scalar1=0.0, scalar2=-1.0,
                        op0=OP.max, op1=OP.add)
                    h = dwork.tile([128, HID], F32, tag="h")
                    nc.vector.tensor_tensor(
                        out=h[:], in0=rm1[:], in1=ex[:], op=OP.add)

                    # p | r = h @ [W2l | W2r]
                    prp = dps2.tile([128, 2 * OUT], F32, tag="pr")
                    for hh in range(2):
                        tph = dps1.tile([128, 128], F32, tag="tph")
                        nc.tensor.transpose(
                            tph[:], h[:, 128 * hh:128 * (hh + 1)],
                            ident_sb[:])
                        hts = mwork.tile([128, 128], F32, tag="hts")
                        nc.scalar.activation(hts[:], tph[:], AF.Copy)
                        nc.tensor.matmul(
                            prp[:], hts[:], w2lr_sb[:, 4 * hh:4 * (hh + 1)],
                            start=(hh == 0), stop=(hh == 1))
                    nc.scalar.activation(pr_sb[:, w, :], prp[:], AF.Copy)
                    nc.sync.dma_start(
                        out=pk_local[128 * w:128 * (w + 1), :],
                        in_=pr_sb[:, w, 0:OUT])
                    if DEBUG:
                        nc.sync.dma_start(
                            out=dagg_d[128 * w:128 * (w + 1), :],
                            in_=mean_sb[:])
                        nc.sync.dma_start(
                            out=dpr_d[128 * w:128 * (w + 1), :],
                            in_=pr_sb[:, w, :])

                obufs = {}
                pend_dense = []
                pw = None
                for c in range(c_total):
                    info = chunk_info[c]
                    if info is None:
                        continue
                    w, s2, par, s_start, s_stop = info
                    o_t = onehot_for(c, opool, obufs)
                    g_t, col = gmap[c]
                    if s_start:
                        pw = pwp.tile([128, D], F32, tag="pw")
                    nc.tensor.matmul(
                        pw[64 * s2:64 * s2 + 64, :],
                        o_t[:, c - (c // G) * G, :],
                        g_t[:, col, 64 * par:64 * par + 64],
                        start=s_start, stop=s_stop)
                    if s_stop:
                        if w not in cur_mean:
                            mtile = mwork.tile([128, D], F32, tag="mean")
                            cur_mean[w] = mtile
                        nc.vector.tensor_scalar(
                            out=cur_mean[w][64 * s2:64 * s2 + 64, :],
                            in0=pw[64 * s2:64 * s2 + 64, :],
                            scalar1=rc_sb[64 * s2:64 * s2 + 64, w:w + 1],
                            scalar2=None, op0=OP.mult)
                    if c in fire:
                        for wf in fire[c]:
                            pend_dense.append(wf)
                            if len(pend_dense) == 3:
                                dense(pend_dense.pop(0))
                for wf in pend_dense:
                    dense(wf)

            # =============== AllGather p + build conv2 table ===============
            nc.gpsimd.collective_compute(
                "AllGather",
                OP.bypass,
                replica_groups=[list(range(N_CORES))],
                ins=[pk_local.opt()],
                outs=[pk_all.opt()],
            )
            nc.sync.dma_start(out=pa_sb[:], in_=pk_all[:])
            if DEBUG:
                nc.sync.dma_start(out=dpa_d[:], in_=pa_sb[:])
            with tc.tile_pool(name="tabp", bufs=2) as tabp:
                npiece = 4
                pb = nblk // npiece
                for pc in range(npiece):
                    tp_ = tabp.tile([128, pb, 16, 2 * OUT], F32, tag="tp")
                    srcv = pa_sb[:, pb * pc:pb * (pc + 1), :].unsqueeze(
                        2).broadcast_to([128, pb, 16, 2 * OUT])
                    nc.vector.tensor_copy(out=tp_[:], in_=srcv)
                    nc.sync.dma_start(
                        out=tab2[:, pb * pc:pb * (pc + 1), :], in_=tp_[:])

            # =================== conv2 ===================
            with (
                tc.tile_pool(name="gpool2", bufs=10) as gpool2,
                tc.tile_pool(name="opool2", bufs=6) as opool2,
                tc.tile_pool(name="pw2p", bufs=4, space="PSUM") as pw2p,
                tc.tile_pool(name="ewk", bufs=6) as ewk,
            ):
                gmap2 = emit_gathers(tab2_flat, D, gpool2, "g2")
                cur_t = {}

                def epi2(w):
                    t = cur_t.pop(w)
                    nc.vector.tensor_tensor(
                        out=t[:], in0=t[:], in1=pr_sb[:, w, OUT:2 * OUT],
                        op=OP.add)
                    nc.vector.tensor_tensor(
                        out=out_sb[:, w, :], in0=t[:], in1=b2_sb[:],
                        op=OP.add)
                    nc.sync.dma_start(
                        out=out_d[128 * w:128 * (w + 1), :],
                        in_=out_sb[:, w, :])

                obufs2 = {}
                pend2 = []
                pw2 = None
                for c in range(c_total):
                    info = chunk_info[c]
                    if info is None:
                        continue
                    w, s2, par, s_start, s_stop = info
                    o_t = onehot_for(c, opool2, obufs2)
                    g_t, col = gmap2[c]
                    if s_start:
                        pw2 = pw2p.tile([128, OUT], F32, tag="pw2")
                    nc.tensor.matmul(
                        pw2[64 * s2:64 * s2 + 64, :],
                        o_t[:, c - (c // G) * G, :],
                        g_t[:, col, 2 * par:2 * par + 2],
                        start=s_start, stop=s_stop)
                    if s_stop:
                        if w not in cur_t:
                            ttile = ewk.tile([128, OUT], F32, tag="t")
                            cur_t[w] = ttile
                        nc.vector.tensor_scalar(
                            out=cur_t[w][64 * s2:64 * s2 + 64, :],
                            in0=pw2[64 * s2:64 * s2 + 64, :],
                            scalar1=rc_sb[64 * s2:64 * s2 + 64, w:w + 1],
                            scalar2=None, op0=OP.mult)
                    if c in fire:
                        for wf in fire[c]:
                            pend2.append(wf)
                            if len(pend2) == 3:
                                epi2(pend2.pop(0))
                for wf in pend2:
                    epi2(wf)

    nc.compile()
    return nc


def make_inputs(plan, x, W1l, W1r, b1, Wskip, bskip, gamma, beta, W2l, W2r,
                b2, n_nodes):
    cp, half, npad, nw = plan["cp"], plan["half"], plan["npad"], plan["nw"]
    xp = np.zeros((npad, D), np.float32)
    xp[:n_nodes] = np.asarray(x, np.float32)
    wc = np.asarray(W1r, np.float32) + np.asarray(Wskip, np.float32)
    bc = np.asarray(b1, np.float32) + np.asarray(bskip, np.float32)
    wcb = np.concatenate([wc, bc[None, :]], axis=0)
    w2lr_full = np.concatenate(
        [np.asarray(W2l, np.float32), np.asarray(W2r, np.float32)], axis=1)
    w2lr = (
        w2lr_full.reshape(2, 128, 2 * OUT).transpose(1, 0, 2)
        .reshape(128, 2 * 2 * OUT).copy()
    )
    iota64 = np.broadcast_to(
        np.arange(64, dtype=np.float32)[None, None, :],
        (128, G, 64)).copy()
    ident = np.eye(128, dtype=np.float32)
    gamma_bc = np.tile(np.asarray(gamma, np.float32)[None, :], (128, 1))
    beta_bc = np.tile(np.asarray(beta, np.float32)[None, :], (128, 1))
    b2_bc = np.tile(np.asarray(b2, np.float32)[None, :], (128, 1))

    common = dict(
        x2=xp.reshape(npad // 2, 2 * D).copy(),
        iota64=iota64, ident=ident,
        wcb=wcb, w1l=np.asarray(W1l, np.float32), w2lr=w2lr,
        gamma_bc=gamma_bc, beta_bc=beta_bc, b2_bc=b2_bc,
    )
    in_maps = []
    for c in range(N_CORES):
        m = dict(common)
        xc_loc = xp[cp * c:cp * (c + 1)]
        xt = np.empty((D + 1, cp), np.float32)
        xt[0:D] = xc_loc.T
        xt[D] = 1.0
        m["xt"] = xt
        m["gidx"] = plan["gidx_tile"][c]
        m["dstf"] = plan["dstf_tile"][c]
        m["rc"] = plan["rc_tile"][c]
        in_maps.append(m)
    return in_maps


_CACHE = {}


def _get_compiled(edge_index, n_nodes):
    key = (edge_index.tobytes()[:512], edge_index.shape, n_nodes)
    if key not in _CACHE:
        plan = make_plan(edge_index, n_nodes)
        nc = build_program(plan)
        _CACHE[key] = (plan, nc)
    return _CACHE[key]


def run(inputs, trace=False):
    x = np.asarray(inputs["x"], np.float32)
    edge_index = np.asarray(inputs["edge_index"], np.int32)
    n_nodes = x.shape[0]
    plan, nc = _get_compiled(edge_index, n_nodes)
    in_maps = make_inputs(
        plan, x, inputs["W1l"], inputs["W1r"], inputs["b1"], inputs["Wskip"],
        inputs["bskip"], inputs["gamma"], inputs["beta"], inputs["W2l"],
        inputs["W2r"], inputs["b2"], n_nodes)
    res = run_bass_kernel_spmd(
        nc, in_maps, list(range(N_CORES)), trace=trace)
    cp = plan["cp"]
    out = np.empty((n_nodes, OUT), np.float32)
    for c in range(N_CORES):
        lo = cp * c
        hi = min(cp * (c + 1), n_nodes)
        out[lo:hi] = res.results[c]["out"][0:hi - lo]
    return out, res


def kernel(**inputs) -> np.ndarray:
    out, _ = run(inputs)
    return out
